# revision 5
# baseline (speedup 1.0000x reference)
"""Trainium2 Bass kernel for nn_CurriculumLearningGuidedDynamicLoss.

Data-parallel over 8 NeuronCores: batch 1024 -> 128 rows/core.
Per core the device computes, in one fused Tile kernel:
  - Pearson stats of (pred, targ): mean/var of pred via bn_stats,
    sum/sumsq of targ via ScalarE accumulate, sum(pred*targ) via the
    fused DVE tensor_tensor_reduce.
  - A 140-bin hann-windowed non-uniform DFT of pred:
    pred is cast to bf16, transposed 128x128 via the PE, and matmul'd
    against host-precomputed bf16 sin/cos tables (hann folded in),
    accumulating s,c in PSUM; ca = s^2 + c^2 is L1-normalized on chip.
Host combines the per-row outputs (tiny: 1024x146) into the scalar loss.
"""
import math
import os
import sys

for _p in ("/opt/trn_rl_repo", "/opt/pypackages"):
    if _p not in sys.path:
        sys.path.insert(0, _p)

import numpy as np
import ml_dtypes

import concourse.bass as bass
import concourse.tile as tile
from concourse import mybir
from concourse.vector_clock import ScopedClock

# ----------------------------------------------------------------------------
# This walrus build accepts only ONE sync-wait on an InstDrain; TileContext's
# final drain carries one wait per live logical proc.  Split the extra waits
# onto individual sync NOPs ahead of a clean drain.
# ----------------------------------------------------------------------------


def _patched_drain_and_barrier(self, tick_clock, wait_clock):
    nc = self.nc
    collector = nc.sync.nop(nofuse=True)
    wait_clock.add_sem_waits(
        collector.ins, ScopedClock({None: tick_clock.global_clock})
    )
    waits = list(collector.ins.sync_info.on_wait or [])
    if len(waits) > 1:
        collector.ins.sync_info.on_wait = waits[:1]
        for w in waits[1:]:
            extra = nc.sync.nop(nofuse=True)
            extra.ins.sync_info = mybir.SyncInfo(on_wait=[w], on_update=[])
    nc.sync.drain()
    nc.all_engine_barrier()
    assert self.sems is not None
    popped = nc._tile_sem_poison_stack.pop()
    assert popped is self._sem_poison
    nc.clear_and_free_semaphores(list(self.sems.allocated().values()))
    nc.all_engine_barrier()


tile.TileContext._drain_and_barrier = _patched_drain_and_barrier


def _split_multiwait_json(data: bytes) -> bytes:
    """walrus in this container accepts one sync-wait per instruction
    (two for EventSemaphore).  Hoist extra waits onto same-engine NoOps
    inserted immediately before the over-subscribed instruction."""
    import json

    mod = json.loads(data)

    def fix_list(insts):
        out = []
        for inst in insts:
            si = inst.get("sync_info")
            waits = (si or {}).get("on_wait") or []
            cap = 2 if inst.get("opcode") == "EventSemaphore" else 1
            if len(waits) > cap:
                keep = waits[-cap:]
                for k, w in enumerate(waits[:-cap]):
                    out.append({
                        "debug": inst.get("debug", 0),
                        "engine": inst["engine"],
                        "ins": [],
                        "name": f"{inst['name']}-xw{k}",
                        "opcode": "NoOp",
                        "outs": [],
                        "sync_info": {"on_update": [], "on_wait": [w]},
                    })
                si["on_wait"] = keep
            out.append(inst)
        insts[:] = out

    def walk(o):
        if isinstance(o, dict):
            if isinstance(o.get("instructions"), list):
                fix_list(o["instructions"])
            for v in o.values():
                walk(v)
        elif isinstance(o, list):
            for v in o:
                walk(v)

    walk(mod)
    return json.dumps(mod).encode()


def _install_json_splitter(nc):
    orig = nc.to_json_bytes

    def patched(*a, **k):
        return _split_multiwait_json(orig(*a, **k))

    nc.to_json_bytes = patched
    return nc

F32 = mybir.dt.float32
BF16 = mybir.dt.bfloat16

NCORES = 8
B = 1024          # full batch
PB = 128          # rows per core
N = 8192          # samples per row
NK = 140          # bpm bins
NCH = 8           # big DMA chunks
CW = N // NCH     # 1024 cols per chunk
NSUB = CW // 128  # 8 transpose subchunks per big chunk

_CACHE = {}


def _build_module():
    nc = bass.Bass()
    pred = nc.declare_dram_parameter("pred", [PB, N], F32, isOutput=False)
    targ = nc.declare_dram_parameter("targ", [PB, N], F32, isOutput=False)
    tabs = nc.declare_dram_parameter("tabs", [PB, 64, 2 * NK], BF16, isOutput=False)
    ident = nc.declare_dram_parameter("ident", [128, 128], BF16, isOutput=False)
    out = nc.declare_dram_parameter("out", [PB, 146], F32, isOutput=True)

    AX = mybir.AxisListType.X
    ALU = mybir.AluOpType
    ACTF = mybir.ActivationFunctionType

    with tile.TileContext(nc) as tc:
        with (
            tc.tile_pool(name="consts", bufs=1) as consts,
            tc.tile_pool(name="big", bufs=1) as big,
            tc.tile_pool(name="psumT", bufs=4, space="PSUM") as psumT_pool,
            tc.tile_pool(name="psumF", bufs=1, space="PSUM") as psumF_pool,
            tc.tile_pool(name="small", bufs=1) as small,
        ):
            ident_sb = consts.tile([128, 128], BF16, tag="ident")
            nc.sync.dma_start(out=ident_sb, in_=ident[:, :])

            pred_t = [big.tile([PB, CW], F32, tag=f"pred{i}", name=f"pred{i}") for i in range(NCH)]
            targ_t = [big.tile([PB, CW], F32, tag=f"targ{i}", name=f"targ{i}") for i in range(NCH)]
            pbf_t = [big.tile([PB, CW], BF16, tag=f"pbf{i}", name=f"pbf{i}") for i in range(NCH)]
            tab_t = [
                big.tile([PB, NSUB, 2 * NK], BF16, tag=f"tab{i}", name=f"tab{i}") for i in range(NCH)
            ]
            predT_t = [
                big.tile([128, 512], BF16, tag=f"predT{i}", name=f"predT{i}") for i in range(2 * NCH)
            ]
            scratchV = big.tile([PB, CW], F32, tag="scratchV")
            scratchA = big.tile([PB, CW], F32, tag="scratchA")

            sxy_p = small.tile([PB, NCH], F32, tag="sxy_p")
            sy_p = small.tile([PB, NCH], F32, tag="sy_p")
            sy2_p = small.tile([PB, NCH], F32, tag="sy2_p")
            bnst = small.tile([PB, 2 * NCH, 6], F32, tag="bnst")
            sq_s = small.tile([PB, NK], F32, tag="sq_s")
            sq_c = small.tile([PB, NK], F32, tag="sq_c")
            ca = small.tile([PB, NK], F32, tag="ca")
            cinv = small.tile([PB, 1], F32, tag="cinv")
            out_sb = small.tile([PB, 146], F32, tag="out_sb")

            psum_dft = psumF_pool.tile([128, 2 * NK], F32, tag="dft")

            for i in range(NCH):
                nc.sync.dma_start(out=pred_t[i], in_=pred[:, i * CW:(i + 1) * CW])
                nc.sync.dma_start(out=targ_t[i], in_=targ[:, i * CW:(i + 1) * CW])
                nc.sync.dma_start(
                    out=tab_t[i], in_=tabs[:, i * NSUB:(i + 1) * NSUB, :]
                )

            for i in range(NCH):
                # bf16 cast for the DFT path (ScalarE)
                nc.scalar.activation(out=pbf_t[i], in_=pred_t[i], func=ACTF.Copy)
                # sum(pred*targ) in one fused DVE pass
                nc.vector.scalar_tensor_tensor(
                    out=scratchV,
                    in0=pred_t[i],
                    scalar=1.0,
                    in1=targ_t[i],
                    op0=ALU.mult,
                    op1=ALU.mult,
                    accum_out=sxy_p[:, i:i + 1],
                )
                # mean/var of pred (DVE bn pipeline, 512-wide subgroups)
                nc.vector.bn_stats(
                    out=bnst[:, 2 * i, :], in_=pred_t[i][:, 0:512]
                )
                nc.vector.bn_stats(
                    out=bnst[:, 2 * i + 1, :], in_=pred_t[i][:, 512:1024]
                )
                # sum(targ), sum(targ^2) on ScalarE with fused accumulate
                nc.scalar.activation(
                    out=scratchA, in_=targ_t[i], func=ACTF.Copy,
                    accum_out=sy_p[:, i:i + 1],
                )
                nc.scalar.activation(
                    out=scratchA, in_=targ_t[i], func=ACTF.Square,
                    accum_out=sy2_p[:, i:i + 1],
                )

            # PE: transpose pred_bf 128x128 blocks, evacuate, then the DFT
            for i in range(NCH):
                for g in range(2):
                    pt = psumT_pool.tile([128, 512], BF16, tag="psumT")
                    for j in range(4):
                        sub = 4 * g + j
                        nc.tensor.transpose(
                            pt[:, 128 * j:128 * (j + 1)],
                            pbf_t[i][:, 128 * sub:128 * (sub + 1)],
                            ident_sb,
                        )
                    nc.vector.tensor_copy(out=predT_t[2 * i + g], in_=pt)
                for sub in range(NSUB):
                    c = NSUB * i + sub
                    g, sl = divmod(sub, 4)
                    nc.tensor.matmul(
                        psum_dft,
                        predT_t[2 * i + g][:, 128 * sl:128 * (sl + 1)],
                        tab_t[i][:, sub, :],
                        start=(c == 0),
                        stop=(c == 63),
                        skip_group_check=True,
                    )

            # reduce stat partials into the output tile
            nc.vector.bn_aggr(out=out_sb[:, 140:142], in_=bnst)
            nc.vector.reduce_sum(out=out_sb[:, 142:143], in_=sy_p, axis=AX)
            nc.vector.reduce_sum(out=out_sb[:, 143:144], in_=sy2_p, axis=AX)
            nc.vector.reduce_sum(out=out_sb[:, 144:145], in_=sxy_p, axis=AX)

            # ca = s^2 + c^2, L1-normalized
            nc.scalar.activation(out=sq_s, in_=psum_dft[:, 0:NK], func=ACTF.Square)
            nc.scalar.activation(out=sq_c, in_=psum_dft[:, NK:2 * NK], func=ACTF.Square)
            nc.vector.tensor_add(ca, sq_s, sq_c)
            nc.vector.reduce_sum(out=out_sb[:, 145:146], in_=ca, axis=AX)
            nc.vector.reciprocal(out=cinv, in_=out_sb[:, 145:146])
            nc.vector.tensor_scalar_mul(out_sb[:, 0:NK], ca, cinv)

            nc.sync.dma_start(out=out[:, :], in_=out_sb)

    return nc


def _host_consts():
    n = np.arange(N, dtype=np.float64)
    bpm = np.arange(40, 180, dtype=np.float64)
    ang = 2.0 * np.pi * np.outer(n, bpm) / 1800.0          # (N, NK)
    hann = np.hanning(N)
    sin_t = np.sin(ang) * hann[:, None]
    cos_t = np.cos(ang) * hann[:, None]
    sinT = sin_t.reshape(64, 128, NK).transpose(1, 0, 2)    # (128, 64, NK)
    cosT = cos_t.reshape(64, 128, NK).transpose(1, 0, 2)
    tabs = np.concatenate([sinT, cosT], axis=2)             # (128, 64, 280)
    tabs = tabs.astype(ml_dtypes.bfloat16)
    ident = np.eye(128, dtype=ml_dtypes.bfloat16)
    return tabs, ident


def kernel(epoch, predicted_rppg, target_ppg, average_hr):
    from concourse.bass_utils import run_bass_kernel_spmd

    if "nc" not in _CACHE:
        _CACHE["nc"] = _install_json_splitter(_build_module())
        _CACHE["consts"] = _host_consts()
    nc = _CACHE["nc"]
    tabs, ident = _CACHE["consts"]

    pred = np.ascontiguousarray(np.asarray(predicted_rppg, dtype=np.float32))
    targ = np.ascontiguousarray(np.asarray(target_ppg, dtype=np.float32))
    hr = np.asarray(average_hr).astype(np.int64)
    ep = int(np.asarray(epoch))

    in_maps = [
        {
            "pred": pred[i * PB:(i + 1) * PB],
            "targ": targ[i * PB:(i + 1) * PB],
            "tabs": tabs,
            "ident": ident,
        }
        for i in range(NCORES)
    ]
    res = run_bass_kernel_spmd(
        nc, in_maps, list(range(NCORES)),
        trace=bool(int(os.environ.get("KBENCH_TRACE", "0"))),
    )
    if res.exec_time_ns is not None:
        _CACHE["exec_time_ns"] = res.exec_time_ns
    outs = np.concatenate(
        [np.asarray(res.results[i]["out"], dtype=np.float64) for i in range(NCORES)],
        axis=0,
    )  # (1024, 146)

    can = outs[:, 0:NK]
    mu_x = outs[:, 140]
    var_x = outs[:, 141]
    sy = outs[:, 142]
    sy2 = outs[:, 143]
    sxy = outs[:, 144]

    # Pearson (population cov/var ratio == the reference formula analytically)
    mu_y = sy / N
    var_y = sy2 / N - mu_y * mu_y
    cov = sxy / N - mu_x * mu_y
    r = cov / np.sqrt(var_x * var_y)
    temporal = np.mean(1.0 - r)

    # frequency losses from the normalized spectrum
    m = can.max(axis=1, keepdims=True)
    logp = can - (m + np.log(np.exp(can - m).sum(axis=1, keepdims=True)))
    ce = -logp[np.arange(B), hr].mean()
    i = np.arange(NK, dtype=np.float64)
    t = np.exp(-((i[None, :] - hr[:, None].astype(np.float64)) ** 2) / 2.0) / (
        math.sqrt(2.0 * math.pi)
    )
    t = np.maximum(t, 1e-15)
    kl = ((np.exp(t) * (t - logp)).sum(axis=1) / NK).mean()

    if ep > 25:
        alpha, beta = 0.05, 2.0
    else:
        alpha = 0.1 * math.pow(0.5, ep / 25.0)
        beta = 1.0 * math.pow(2.0, ep / 25.0)

    return np.float32(alpha * temporal + beta * (ce + kl))
